# revision 2
# baseline (speedup 1.0000x reference)
"""Multi-head attention Trainium2 kernel (8 NeuronCores, tensor-parallel over heads).

Problem: B=2, S=2048, D=1024, H=16, Dh=64 fp32 MHA (QKV proj -> softmax(QK^T/8)V -> out proj).

Sharding: each core owns 2 heads (column-parallel QKV, row-parallel out-proj).
Per core:
  - QKV computed transposed: QT/KT/VTc = w_c^T @ x^T  [128 headcols, 4096 seq]
    (softmax scale 1/8 folded into wq/bq on the host)
  - V transposed back per 128-seq chunk on the PE, augmented with a ones
    column so attn@V also produces the softmax denominators.
  - scores^T per (batch, head, 512-s-tile, 128-t-chunk) = K^T-slice.T @ Q^T-slice,
    the two heads row-packed (K=64 each, partition bases 0/64) to run concurrently.
  - exp on ScalarE directly from PSUM, accumulate attn@V over t-chunks,
    normalize with reciprocal + gpsimd partition-broadcast.
  - out-proj: out^T partial = wo_c m-tile.T @ O^T stripe; host sums the 8
    partials, adds bo, transposes back.
All matmuls run as float32r (fp32 data, PE rounds to ~tf32 precision, bf16-rate).
"""

import numpy as np

B = 2
S = 2048
D = 1024
H = 16
DH = 64
NCORES = 8
HPC = H // NCORES          # heads per core = 2
CW = HPC * DH              # head columns per core = 128
BS = B * S                 # 4096
NCH = 8                    # 512-wide column chunks over B*S
KT_TILES = D // 128        # 8 contraction tiles for projections
TCH = S // 128             # 16 t-chunks per batch
STILES = S // 512          # 4 s-tiles per batch
MT = D // 128              # 8 out-proj m-tiles

_CACHE = {}


def _build():
    import concourse.mybir as mybir
    from concourse import bacc
    from concourse.tile import TileContext
    from concourse.masks import make_identity

    f32 = mybir.dt.float32
    f32r = mybir.dt.float32r
    Exp = mybir.ActivationFunctionType.Exp

    nc = bacc.Bacc(trn_type="TRN2")
    xT = nc.dram_tensor("xT", (D, BS), f32r, kind="ExternalInput")
    wq = nc.dram_tensor("wq", (D, CW), f32r, kind="ExternalInput")
    wk = nc.dram_tensor("wk", (D, CW), f32r, kind="ExternalInput")
    wv = nc.dram_tensor("wv", (D, CW), f32r, kind="ExternalInput")
    wo = nc.dram_tensor("wo", (CW, D), f32r, kind="ExternalInput")
    bq = nc.dram_tensor("bq", (CW, 1), f32, kind="ExternalInput")
    bk = nc.dram_tensor("bk", (CW, 1), f32, kind="ExternalInput")
    bv = nc.dram_tensor("bv", (CW, 1), f32, kind="ExternalInput")
    outT = nc.dram_tensor("outT", (D, BS), f32, kind="ExternalOutput")

    xT4 = xT.rearrange("(kt p) s -> p kt s", p=128)
    wq4 = wq.rearrange("(kt p) m -> p kt m", p=128)
    wk4 = wk.rearrange("(kt p) m -> p kt m", p=128)
    wv4 = wv.rearrange("(kt p) m -> p kt m", p=128)
    wo4 = wo.rearrange("p (mt m) -> p mt m", m=128)

    with nc.allow_low_precision(reason="fp32r attention"), TileContext(nc) as tc:
        with tc.tile_pool(name="const", bufs=1) as const, \
             tc.tile_pool(name="wp", bufs=1) as wp, \
             tc.tile_pool(name="qk", bufs=1) as qk, \
             tc.tile_pool(name="vap", bufs=1) as vap, \
             tc.tile_pool(name="xs", bufs=2) as xs, \
             tc.tile_pool(name="vts", bufs=3) as vts, \
             tc.tile_pool(name="es", bufs=4) as es, \
             tc.tile_pool(name="ob", bufs=3) as ob, \
             tc.tile_pool(name="pmm", bufs=3, space="PSUM") as pmm, \
             tc.tile_pool(name="ptr", bufs=1, space="PSUM") as ptr, \
             tc.tile_pool(name="pso", bufs=2, space="PSUM") as pso, \
             tc.tile_pool(name="pout", bufs=2, space="PSUM") as pout:

            ident = const.tile([128, 128], f32)
            make_identity(nc, ident)
            ones = const.tile([128, 1], f32)
            nc.vector.memset(ones, 1.0)

            wq_t = wp.tile([128, KT_TILES, CW], f32r)
            wk_t = wp.tile([128, KT_TILES, CW], f32r)
            wv_t = wp.tile([128, KT_TILES, CW], f32r)
            wo_t = wp.tile([128, MT, 128], f32r)
            nc.sync.dma_start(out=wq_t, in_=wq4)
            nc.sync.dma_start(out=wk_t, in_=wk4)
            nc.sync.dma_start(out=wv_t, in_=wv4)
            nc.sync.dma_start(out=wo_t, in_=wo4)
            bq_t = wp.tile([CW, 1], f32)
            bk_t = wp.tile([CW, 1], f32)
            bv_t = wp.tile([CW, 1], f32)
            nc.sync.dma_start(out=bq_t, in_=bq[:, :])
            nc.sync.dma_start(out=bk_t, in_=bk[:, :])
            nc.sync.dma_start(out=bv_t, in_=bv[:, :])

            QT = qk.tile([128, BS], f32r)
            KT = qk.tile([128, BS], f32r)
            # Vaug chunks: per (b,h) pair, 16 chunks of [128 t, 64 V + 1 ones]
            VA = vap.tile([128, B * HPC * TCH, DH + 1], f32r)

            # ---- Phase 1: QKV projections (+ V transpose) ----
            for n in range(NCH):
                cs = slice(n * 512, (n + 1) * 512)
                xq = xs.tile([128, KT_TILES, 512], f32r, tag="xq", name=f"xq{n}")
                nc.sync.dma_start(out=xq, in_=xT4[:, :, cs])
                for wt, bias, dst in ((wq_t, bq_t, QT), (wk_t, bk_t, KT)):
                    pm = pmm.tile([128, 512], f32, tag="pmm", name=f"pm{n}{dst.name}")
                    for kt in range(KT_TILES):
                        nc.tensor.matmul(pm, wt[:, kt], xq[:, kt],
                                         start=(kt == 0), stop=(kt == KT_TILES - 1))
                    nc.vector.tensor_scalar_add(dst[:, cs], pm, bias)
                pm = pmm.tile([128, 512], f32, tag="pmm", name=f"pmv{n}")
                for kt in range(KT_TILES):
                    nc.tensor.matmul(pm, wv_t[:, kt], xq[:, kt],
                                     start=(kt == 0), stop=(kt == KT_TILES - 1))
                vtc = vts.tile([128, 512], f32, tag="vtc", name=f"vtc{n}")
                nc.vector.tensor_scalar_add(vtc, pm, bv_t)
                b = n // STILES
                for sc in range(4):
                    chunk = (n % STILES) * 4 + sc
                    for h in range(HPC):
                        hsl = slice(h * DH, (h + 1) * DH)
                        pt = ptr.tile([128, DH], f32, tag="ptr", name=f"pt{n}{sc}{h}")
                        nc.tensor.transpose(pt, vtc[hsl, sc * 128:(sc + 1) * 128],
                                            ident[hsl, hsl])
                        va = VA[:, (b * HPC + h) * TCH + chunk, :]
                        nc.vector.tensor_copy(va[:, 0:DH], pt)
                        nc.vector.tensor_copy(va[:, DH:DH + 1], ones)

            # ---- Phase 2: attention + out projection ----
            for b in range(B):
                for st in range(STILES):
                    ssl = slice(b * S + st * 512, b * S + (st + 1) * 512)
                    OT = ob.tile([128, 512], f32r, tag="OT", name=f"OT{b}{st}")
                    psO = [None, None]
                    for tch in range(TCH):
                        psS = [None, None]
                        for h in range(HPC):
                            hsl = slice(h * DH, (h + 1) * DH)
                            tsl = slice(b * S + tch * 128, b * S + (tch + 1) * 128)
                            psS[h] = pmm.tile([128, 512], f32, tag="pmm",
                                              name=f"psS{b}{st}{tch}{h}")
                            nc.tensor.matmul(psS[h], KT[hsl, tsl], QT[hsl, ssl],
                                             start=True, stop=True)
                        for h in range(HPC):
                            if tch == 0:
                                psO[h] = pso.tile([DH + 1, 512], f32, tag="pso",
                                                  name=f"psO{b}{st}{h}")
                            e = es.tile([128, 512], f32r, tag="e", name=f"e{b}{st}{tch}{h}")
                            nc.scalar.activation(e, psS[h], Exp)
                            nc.tensor.matmul(psO[h], VA[:, (b * HPC + h) * TCH + tch, :], e,
                                             start=(tch == 0), stop=(tch == TCH - 1))
                    for h in range(HPC):
                        r = ob.tile([1, 512], f32, tag="r", name=f"r{b}{st}{h}")
                        nc.vector.reciprocal(r, psO[h][DH:DH + 1, :])
                        bc = ob.tile([DH, 512], f32, tag="bc", name=f"bc{b}{st}{h}")
                        nc.gpsimd.partition_broadcast(bc, r)
                        nc.vector.tensor_mul(OT[h * DH:(h + 1) * DH, :], psO[h][0:DH, :], bc)
                    for mt in range(MT):
                        po = pout.tile([128, 512], f32, tag="pout", name=f"po{b}{st}{mt}")
                        nc.tensor.matmul(po, wo_t[:, mt], OT, start=True, stop=True)
                        oc = ob.tile([128, 512], f32, tag="oc", name=f"oc{b}{st}{mt}")
                        nc.vector.tensor_copy(oc, po)
                        nc.sync.dma_start(out=outT[mt * 128:(mt + 1) * 128, ssl], in_=oc)
    nc.compile()
    return nc


def _prepare_in_maps(x, wq, bq, wk, bk, wv, bv, wo):
    xT = np.ascontiguousarray(x.reshape(BS, D).T).astype(np.float32)
    scale = np.float32(1.0 / np.sqrt(DH))
    in_maps = []
    for c in range(NCORES):
        cols = slice(c * CW, (c + 1) * CW)
        in_maps.append({
            "xT": xT,
            "wq": np.ascontiguousarray(wq[:, cols] * scale),
            "wk": np.ascontiguousarray(wk[:, cols]),
            "wv": np.ascontiguousarray(wv[:, cols]),
            "wo": np.ascontiguousarray(wo[cols, :]),
            "bq": np.ascontiguousarray((bq[cols] * scale).reshape(CW, 1)),
            "bk": np.ascontiguousarray(bk[cols].reshape(CW, 1)),
            "bv": np.ascontiguousarray(bv[cols].reshape(CW, 1)),
        })
    return in_maps


def _run(inputs, trace=False):
    from concourse.bass_utils import run_bass_kernel_spmd

    if "nc" not in _CACHE:
        _CACHE["nc"] = _build()
    nc = _CACHE["nc"]
    in_maps = _prepare_in_maps(
        np.asarray(inputs["x"], dtype=np.float32),
        np.asarray(inputs["wq"], dtype=np.float32),
        np.asarray(inputs["bq"], dtype=np.float32),
        np.asarray(inputs["wk"], dtype=np.float32),
        np.asarray(inputs["bk"], dtype=np.float32),
        np.asarray(inputs["wv"], dtype=np.float32),
        np.asarray(inputs["bv"], dtype=np.float32),
        np.asarray(inputs["wo"], dtype=np.float32),
    )
    res = run_bass_kernel_spmd(nc, in_maps, core_ids=list(range(NCORES)), trace=trace)
    acc = res.results[0]["outT"].astype(np.float32)
    for c in range(1, NCORES):
        acc = acc + res.results[c]["outT"]
    out = acc.T + np.asarray(inputs["bo"], dtype=np.float32)
    return out.reshape(B, S, D).astype(np.float32), res


def kernel(**inputs):
    out, _ = _run(inputs, trace=False)
    return out
